# revision 83
# baseline (speedup 1.0000x reference)
"""Causal GQA self-attention (RoPE) Trainium2 Bass kernel, 8-core SPMD.

Sharding: core c -> (b = c//4, g = c%4).  Data-parallel over batch B=2,
tensor-parallel over the 4 KV groups (4 query heads + 1 KV head each).
Each core computes a partial output y_bg = attn_out_g @ Wo[:, g-block].T
for its batch; the host sums the 4 group partials per batch (row-parallel
linear unshard).

v2 layout (all matmuls bf16, f32 PSUM accumulation):
  xT is transposed on the HOST and DMA'd as [128, 16, L] bf16 (no PE
  transposes).  DMA issue order matches compute order so the PE starts
  ~6us in.  Projection chunks (512 queries) and attention chunks are
  interleaved so the ACT engine's exp work overlaps projection matmuls.
  Attention runs a 2-head, lookahead-2 software pipeline per chunk:
  S-matmul groups run two key-tiles ahead of their exp consumers, which
  hides the ACT exp latency that dominated the v1 stalls.  The causal
  mask is applied with a PE matmul (identity x mask-const accumulated
  into PSUM before the S matmul) instead of a DVE multiply, keeping the
  softmax critical path PE->ACT->PE only.  Output projection runs last
  with 4 rotating PSUM banks; PSUM->SBUF copies run on the ACT engine
  and y tiles stream out per 128x512 block.
"""

import math
import sys

import numpy as np

try:
    import concourse.bass as bass  # noqa: F401
except ImportError:  # pragma: no cover
    sys.path.insert(0, "/opt/trn_rl_repo")
    import concourse.bass as bass  # noqa: F401

import ml_dtypes

import concourse.bacc as bacc
import concourse.bass_isa as bass_isa
import concourse.mybir as mybir
import concourse.tile as tile
from concourse.bass_utils import run_bass_kernel_spmd

BF16 = ml_dtypes.bfloat16
F32 = np.float32

B, L, D = 2, 2048, 2048
HD = 128          # head dim
NHL = 4           # query heads per core (one KV group)
P = 128
NDT = D // P      # 16 d-tiles
NKT = L // P      # 16 key tiles
NLC = L // 512    # 4 512-wide l chunks
SM_SCALE = 1.0 / math.sqrt(HD)
MASK_NEG = -30000.0

_BF = mybir.dt.bfloat16
_F32 = mybir.dt.float32
_EXP = mybir.ActivationFunctionType.Exp
_COPY = mybir.ActivationFunctionType.Copy


def build_nc():
    nc = bacc.Bacc("TRN2", target_bir_lowering=False, debug=False,
                   enable_asserts=False)

    xT_d = nc.dram_tensor("xT", [P, NDT, L], _BF, kind="ExternalInput").ap()
    wq_d = nc.dram_tensor("wq", [P, NHL, NDT, 128], _BF,
                          kind="ExternalInput").ap()
    wk_d = nc.dram_tensor("wk", [P, NDT, 128], _BF, kind="ExternalInput").ap()
    wv_d = nc.dram_tensor("wv", [P, NDT, 128], _BF, kind="ExternalInput").ap()
    wo_d = nc.dram_tensor("wo", [P, NHL, L], _BF, kind="ExternalInput").ap()
    cos_d = nc.dram_tensor("cosT", [P, L], _BF, kind="ExternalInput").ap()
    sin_d = nc.dram_tensor("sinT", [P, L], _BF, kind="ExternalInput").ap()
    perm_d = nc.dram_tensor("perm", [P, P], _BF, kind="ExternalInput").ap()
    ones_d = nc.dram_tensor("ones", [P, P], _BF, kind="ExternalInput").ap()
    msk_d = nc.dram_tensor("msk", [P, P], _BF, kind="ExternalInput").ap()
    id_d = nc.dram_tensor("ident", [P, P], _BF, kind="ExternalInput").ap()
    y_d = nc.dram_tensor("y", [L, D], _F32, kind="ExternalOutput").ap()

    with tile.TileContext(nc) as tc:
        _body(nc, tc, xT_d, wq_d, wk_d, wv_d, wo_d, cos_d, sin_d,
              perm_d, ones_d, msk_d, id_d, y_d)
    nc.compile()
    return nc


def _proj_segment(nc, tc, pa, wsb, lc, xT, wq_sb, wk_sb, wv_sb,
                  cos_sb, sin_sb, perm_sb, id_sb, qT, kT, vn,
                  deferred=None):
    """Q/K/V projections + RoPE for one 512-query chunk.

    Uses only PSUM banks 0-3 (prj x2, qrot, vtp) so the previous
    attention chunk's po/ps banks (4-7) stay untouched: its deferred
    normalization chain (``deferred``) is emitted after the first
    projection group and overlaps this segment's matmuls instead of
    stalling the pool-open."""
    ls = slice(lc * 512, (lc + 1) * 512)

    def w_sl(et, d_):
        if et < 4:
            return wq_sb[:, et, d_, :]
        if et == 4:
            return wk_sb[:, d_, :]
        return wv_sb[:, d_, :]

    def epilogue(et, prj):
        qs = wsb.tile([P, 512], _BF, tag="qs", bufs=3, name=f"qs_{lc}_{et}")
        nc.vector.tensor_copy(qs[:], prj[:])
        if et == 5:
            vtp = pa.tile([P, 512], _BF, tag="vtp", bufs=1,
                          name=f"vtp_{lc}")
            for j in range(4):
                nc.tensor.matmul(vtp[:, j * P:(j + 1) * P],
                                 qs[:, j * P:(j + 1) * P], id_sb[:],
                                 is_transpose=True, skip_group_check=True)
            nc.vector.tensor_copy(vn[:, lc * 4:lc * 4 + 4, :],
                                  vtp[:].rearrange("p (a b) -> p a b", a=4))
        else:
            qrot = pa.tile([P, 512], _F32, tag="qrot", bufs=1,
                           name=f"qrot_{lc}_{et}")
            nc.tensor.matmul(qrot[:], perm_sb[:], qs[:], start=True,
                             stop=True)
            tt = wsb.tile([P, 512], _BF, tag="tt", bufs=2,
                          name=f"tt_{lc}_{et}")
            nc.vector.tensor_mul(tt[:], qs[:], cos_sb[:, ls])
            uu = wsb.tile([P, 512], _BF, tag="uu", bufs=2,
                          name=f"uu_{lc}_{et}")
            nc.vector.tensor_mul(uu[:], qrot[:], sin_sb[:, ls])
            dest = qT[:, et, ls] if et < 4 else kT[:, ls]
            nc.vector.tensor_add(dest, tt[:], uu[:])

    ets = (4, 5, 0, 1, 2, 3)                       # k, v, then 4 q heads
    for ei, et in enumerate(ets):
        if ei == 1 and deferred:
            for fn in deferred:
                fn()
            deferred = None
        prj = pa.tile([P, 512], _F32, tag="prj", bufs=2,
                      name=f"prj_{lc}_{et}")
        for dti in range(NDT):
            nc.tensor.matmul(prj[:], w_sl(et, dti), xT[:, dti, ls],
                             start=(dti == 0), stop=(dti == NDT - 1))
        epilogue(et, prj)


def _op_group(nc, pool, wsb, lt, mc, oT, wo_sb, y_d, bufs, on_act=False):
    """One output-projection PSUM group: 4 head-matmuls -> copy -> DMA.

    The PSUM->SBUF copy runs on DVE by default; ``on_act`` routes it to
    the ACT engine for windows where DVE is serialized on the softmax
    normalization chain (pair drains, final phase)."""
    py = pool.tile([P, 512], _F32, tag="py", bufs=bufs,
                   name=f"py_{lt}_{mc}")
    for h in range(NHL):
        nc.tensor.matmul(py[:], oT[:, h, lt * P:(lt + 1) * P],
                         wo_sb[:, h, mc * 512:(mc + 1) * 512],
                         start=(h == 0), stop=(h == NHL - 1))
    ysb = wsb.tile([P, 512], _F32, tag="ysb", bufs=4, name=f"ysb_{lt}_{mc}")
    if on_act:
        nc.scalar.activation(ysb[:], py[:], _COPY)
    else:
        nc.vector.tensor_copy(ysb[:], py[:])
    nc.sync.dma_start(y_d[lt * P:(lt + 1) * P, mc * 512:(mc + 1) * 512],
                      ysb[:])


def _attn_chunk(nc, tc, pb, wsb, qi, qT, kT, vn, oT, ones_sb, msk_sb,
                id_sb, op_iter, op_args, tail_ops=()):
    """Causal attention for one 512-query chunk, all 4 heads.

    Two heads run in a software pipeline over key tiles so the PE never
    waits on the ACT exp of the tile it is about to consume.  When
    ``op_iter`` is set, one output-projection group of the previous
    chunk is interleaved per round as additional exp-latency cover
    (lookahead drops to 1 to fit PSUM: sc3+po2+ps2+py1 banks).
    """
    q0 = qi * 512
    nvis = 4 * qi
    nkt = nvis + 4
    look = 1
    sc_bufs = 4 if qi == 0 else 2     # chunk 0 has spare banks (no ps/py)
    ops = list(op_iter) if op_iter is not None else []
    deferred = []
    # chunks 1-2 compute softmax denominators on the idle GPSIMD engine
    # (partition reduces of es) instead of PE ones-matmuls; chunk 0 is
    # small and all-GPSIMD on chunk 3 would backlog past the final
    # phase, so chunk 3 splits: GPSIMD for key tiles 0-7, PE-ones for
    # the rest, merged with one DVE add before the reciprocal
    use_gp = qi in (0, 1, 2)
    gp_split = 12 if qi == 3 else 0

    def gp_kt(kt):
        return use_gp or kt < gp_split

    # in-round OP copies default to DVE (ACT rounds stay exp-only, which
    # widens the exp-latency margin); right after a pair boundary DVE
    # still owes the normalization chain, so the next two copies route
    # to ACT instead
    act_boost = [0]

    def op_left():
        return len(ops)

    def emit_op(on_act=None):
        if ops:
            lt, mc = ops.pop(0)
            if on_act is None:
                if act_boost[0] > 0:
                    act_boost[0] -= 1
                    on_act = True
                else:
                    on_act = False
            _op_group(nc, pb, wsb, lt, mc, *op_args, bufs=2, on_act=on_act)

    for pair in ((0, 1), (2, 3)):
        # po/ps tiles are allocated lazily (at the first consumer round)
        # so the sc/py tags claim the low PSUM banks: the next segment's
        # pool then reuses early-freed banks first instead of WAR-waiting
        # on the pair-end reciprocal/normalization reads of po/ps.
        po = {}
        ps = {}
        acc = {}
        es = {}
        if gp_kt(0):
            for h in pair:
                # bf16 so the DVE accumulation adds run in 2x mode
                acc[h] = wsb.tile([P, 512], _BF, tag="acc", bufs=4,
                                  name=f"acc_{qi}_{h}")

        def get_po(h):
            if h not in po:
                po[h] = pb.tile([P, 512], _F32, tag="po",
                                bufs=(4 if (use_gp or qi == 0) else 2),
                                name=f"po_{qi}_{h}")
            return po[h]

        def get_ps(h):
            if h not in ps:
                ps[h] = pb.tile([P, 512], _F32, tag="ps", bufs=2,
                                name=f"ps_{qi}_{h}")
            return ps[h]

        def emit_s(h, kt):
            off = max(0, (kt - nvis) * P)
            cs = slice(off, 512)
            sc = pb.tile([P, 512], _F32, tag="sc", bufs=sc_bufs,
                         name=f"sc_{qi}_{h}_{kt}")
            ktile = kT[:, kt * P:(kt + 1) * P]
            qtile = lambda o: qT[:, h, q0 + o:q0 + 512]
            if kt >= nvis:
                # diagonal tile: mask const first, S accumulates on top
                nc.tensor.matmul(sc[:, off:off + P], id_sb[:], msk_sb[:],
                                 start=True, stop=False,
                                 skip_group_check=True)
                nc.tensor.matmul(sc[:, off:off + P], ktile,
                                 qT[:, h, q0 + off:q0 + off + P],
                                 start=False, stop=True,
                                 skip_group_check=True)
                if off + P < 512:
                    nc.tensor.matmul(sc[:, off + P:512], ktile,
                                     qtile(off + P), start=True, stop=True,
                                     skip_group_check=True)
            else:
                nc.tensor.matmul(sc[:, cs], ktile, qtile(off),
                                 start=True, stop=True,
                                 skip_group_check=True)
            e = wsb.tile([P, 512], _BF, tag="es", bufs=10,
                         name=f"es_{qi}_{h}_{kt}")
            nc.scalar.activation(e[:, cs], sc[:, cs], _EXP, scale=SM_SCALE)
            if gp_kt(kt):
                # partition all-reduce of this key-tile's exp sums on the
                # otherwise idle GPSIMD engine, accumulated over key
                # tiles on DVE; replaces the PE ones-matmul
                gsum = wsb.tile([P, 512], _BF, tag="gsum", bufs=4,
                                name=f"gsum_{qi}_{h}_{kt}")
                nc.gpsimd.partition_all_reduce(
                    gsum[:, cs], e[:, cs], channels=P,
                    reduce_op=bass_isa.ReduceOp.add)
                if kt == 0:
                    nc.vector.tensor_copy(acc[h][:], gsum[:])
                else:
                    nc.vector.tensor_add(acc[h][:, cs], acc[h][:, cs],
                                         gsum[:, cs])
            es[(h, kt)] = e

        def emit_c(h, kt):
            off = max(0, (kt - nvis) * P)
            cs = slice(off, 512)
            e = es.pop((h, kt))
            poh = get_po(h)
            if not gp_kt(kt):
                nc.tensor.matmul(get_ps(h)[:, cs], ones_sb[:], e[:, cs],
                                 start=(kt == gp_split),
                                 stop=(kt == nkt - 1),
                                 skip_group_check=True)
            nc.tensor.matmul(poh[:, cs], vn[:, kt, :], e[:, cs],
                             start=(kt == 0), stop=(kt == nkt - 1),
                             skip_group_check=True)

        if use_gp:
            # no pool-WAR at this chunk's end (normalization is deferred
            # and denominators live in SBUF): spread groups over rounds
            reserve = 3 if pair[0] == 0 else 0
        elif qi == NLC - 1:
            # keep 7 groups to cover the in-place normalization chain
            # that gates the tail output projection
            reserve = 8 if pair[0] == 0 else 8
        else:
            reserve = 6 if pair[0] == 0 else 3
        for kt in range(nkt + look):
            if kt < nkt:
                for h in pair:
                    emit_s(h, kt)
            if op_left() > reserve:
                emit_op()
            if kt >= look:
                for h in pair:
                    emit_c(h, kt - look)

        def normalize(h, poh, psh, acch):
            def fn():
                rec = wsb.tile([P, 512], _F32, tag="rec", bufs=2,
                               name=f"rec_{qi}_{h}")
                if acch is not None and psh is not None:
                    nc.vector.tensor_add(acch[:], acch[:], psh[:])
                den = acch if acch is not None else psh
                nc.vector.reciprocal(rec[:], den[:])
                nc.vector.tensor_mul(oT[:, h, q0:q0 + 512], poh[:], rec[:])
            return fn

        if pair[0] == 0 or qi == NLC - 1:
            for h in pair:
                normalize(h, po[h], ps.get(h), acc.get(h))()
        else:
            # pair 2's normalization chain is deferred into the next
            # segment (which only touches PSUM banks 0-3) so the pool
            # boundary doesn't stall on it
            for h in pair:
                deferred.append(normalize(h, po[h], ps.get(h),
                                          acc.get(h)))
        # emit held-back OP groups under the pair-drain window so the
        # PE stays busy while DVE/ACT drain the pool's last reads
        if not use_gp:
            keep = 3 if pair[0] == 0 else 0
            if qi == NLC - 1 and pair[0] == 0:
                keep = 8
            while op_left() > keep:
                emit_op(on_act=True)
        act_boost[0] = 3

    # last chunk: its own output projection runs here inside the same
    # pool (no pool boundary to WAR-stall on); the final group drains
    # its copy on both engines in halves to shorten the end tail
    for i, (lt, mc) in enumerate(tail_ops):
        oT_, wo_sb_, y_d_ = op_args
        if i == len(tail_ops) - 1:
            py = pb.tile([P, 512], _F32, tag="py", bufs=2, name="py_last")
            for h in range(NHL):
                nc.tensor.matmul(py[:], oT_[:, h, lt * P:(lt + 1) * P],
                                 wo_sb_[:, h, mc * 512:(mc + 1) * 512],
                                 start=(h == 0), stop=(h == NHL - 1))
            for sv in range(2):
                svs = slice(sv * 256, (sv + 1) * 256)
                ysb = wsb.tile([P, 256], _F32, tag="ysl", bufs=2,
                               name=f"ysl_{sv}")
                if sv == 0:
                    nc.scalar.activation(ysb[:], py[:, svs], _COPY)
                else:
                    nc.vector.tensor_copy(ysb[:], py[:, svs])
                nc.sync.dma_start(
                    y_d_[lt * P:(lt + 1) * P,
                         mc * 512 + sv * 256:mc * 512 + (sv + 1) * 256],
                    ysb[:])
        else:
            _op_group(nc, pb, wsb, lt, mc, *op_args, bufs=2,
                      on_act=(i % 2 == 0))

    return deferred


def _body(nc, tc, xT_d, wq_d, wk_d, wv_d, wo_d, cos_d, sin_d,
          perm_d, ones_d, msk_d, id_d, y_d):
    from contextlib import ExitStack
    ctx = ExitStack()
    with ctx:
        pp = ctx.enter_context(tc.tile_pool(name="persist", bufs=1))
        wsb = ctx.enter_context(tc.tile_pool(name="wsb", bufs=2))

        xT = pp.tile([P, NDT, L], _BF, tag="xT")
        wq_sb = pp.tile([P, NHL, NDT, 128], _BF, tag="wq")
        wk_sb = pp.tile([P, NDT, 128], _BF, tag="wk")
        wv_sb = pp.tile([P, NDT, 128], _BF, tag="wv")
        wo_sb = pp.tile([P, NHL, L], _BF, tag="wo")
        cos_sb = pp.tile([P, L], _BF, tag="cos")
        sin_sb = pp.tile([P, L], _BF, tag="sin")
        perm_sb = pp.tile([P, P], _BF, tag="perm")
        ones_sb = pp.tile([P, P], _BF, tag="ones")
        msk_sb = pp.tile([P, P], _BF, tag="msk")
        id_sb = pp.tile([P, P], _BF, tag="ident")
        qT = pp.tile([P, NHL, L], _BF, tag="qT")
        kT = pp.tile([P, L], _BF, tag="kT")
        vn = pp.tile([P, NKT, 128], _BF, tag="vn")
        oT = pp.tile([P, NHL, L], _BF, tag="oT")

        # DMA issue order tracks compute order (sync-engine DMAs are
        # FIFO and hold the SP sequencer while waiting on data).
        nc.sync.dma_start(wk_sb[:], wk_d[:])
        nc.sync.dma_start(xT[:, 0:4, 0:512], xT_d[:, 0:4, 0:512])
        nc.sync.dma_start(xT[:, 4:8, 0:512], xT_d[:, 4:8, 0:512])
        nc.sync.dma_start(xT[:, 8:16, 0:512], xT_d[:, 8:16, 0:512])
        nc.sync.dma_start(wv_sb[:], wv_d[:])
        nc.sync.dma_start(id_sb[:], id_d[:])
        nc.sync.dma_start(perm_sb[:], perm_d[:])
        nc.sync.dma_start(cos_sb[:], cos_d[:])
        nc.sync.dma_start(sin_sb[:], sin_d[:])
        for hq in range(4):
            nc.sync.dma_start(wq_sb[:, hq], wq_d[:, hq])
        nc.sync.dma_start(ones_sb[:], ones_d[:])
        nc.sync.dma_start(msk_sb[:], msk_d[:])
        for lc in range(1, NLC):
            ls = slice(lc * 512, (lc + 1) * 512)
            nc.sync.dma_start(xT[:, :, ls], xT_d[:, :, ls])
        nc.sync.dma_start(wo_sb[:], wo_d[:])

        # interleaved projection / attention chunks; attention chunk qi
        # also consumes the output-projection groups of chunk qi-1
        op_args = (oT, wo_sb, y_d)
        deferred = []
        for lc in range(NLC):
            with tc.tile_pool(name=f"pa{lc}", bufs=1, space="PSUM") as pa:
                _proj_segment(nc, tc, pa, wsb, lc, xT, wq_sb, wk_sb,
                              wv_sb, cos_sb, sin_sb, perm_sb, id_sb,
                              qT, kT, vn, deferred=deferred)
            op_iter = None
            if lc > 0:
                op_iter = iter([(lt, mc) for lt in range(4 * (lc - 1),
                                                        4 * lc)
                                for mc in range(4)])
            tail = ()
            if lc == NLC - 1:
                tail = [(lt, mc) for lt in range(12, 16)
                        for mc in range(4)]
            with tc.tile_pool(name=f"pb{lc}", bufs=1, space="PSUM") as pb:
                deferred = _attn_chunk(nc, tc, pb, wsb, lc, qT, kT, vn,
                                       oT, ones_sb, msk_sb, id_sb,
                                       op_iter, op_args, tail_ops=tail)


def host_constants():
    inv = (1.0 / (10000.0 ** (np.arange(0, HD, 2, dtype=np.float32) / HD))
           ).astype(np.float32)
    t = np.arange(L, dtype=np.float32)
    freqs = t[:, None] * inv[None, :]                    # [L, 64]
    emb = np.concatenate([freqs, freqs], axis=-1)        # [L, 128]
    cosT = np.ascontiguousarray(np.cos(emb).T).astype(BF16)
    sinT = np.ascontiguousarray(np.sin(emb).T).astype(BF16)
    perm = np.zeros((P, P), dtype=F32)
    for i in range(64):
        perm[i + 64, i] = -1.0      # qrot[d] = -q[d+64],  d < 64
        perm[i, i + 64] = 1.0       # qrot[d] =  q[d-64],  d >= 64
    ones = np.ones((P, P), dtype=F32)
    # msk[k, t] = MASK_NEG where key k > query t (strict upper part per
    # diagonal 128-block); added into PSUM before the S matmul.
    msk = np.where(np.arange(P)[:, None] > np.arange(P)[None, :],
                   MASK_NEG, 0.0).astype(F32)
    ident = np.eye(P, dtype=F32)
    return {
        "cosT": cosT, "sinT": sinT,
        "perm": perm.astype(BF16), "ones": ones.astype(BF16),
        "msk": msk.astype(BF16), "ident": ident.astype(BF16),
    }


def make_in_map(consts, x, Wq, Wk, Wv, Wo, b, g):
    qs = slice(g * 512, (g + 1) * 512)
    kvs = slice(g * 128, (g + 1) * 128)
    wq = np.ascontiguousarray(
        Wq[qs].T.reshape(NDT, P, NHL, 128).transpose(1, 2, 0, 3)
    ).astype(BF16)
    wk = np.ascontiguousarray(
        Wk[kvs].T.reshape(NDT, P, 128).transpose(1, 0, 2)).astype(BF16)
    wv = np.ascontiguousarray(
        Wv[kvs].T.reshape(NDT, P, 128).transpose(1, 0, 2)).astype(BF16)
    wo = np.ascontiguousarray(
        Wo[:, qs].T.reshape(NHL, P, D).transpose(1, 0, 2)).astype(BF16)
    xT = np.ascontiguousarray(
        x[b].T.reshape(NDT, P, L).transpose(1, 0, 2)).astype(BF16)
    return {
        "xT": xT,
        "wq": wq, "wk": wk, "wv": wv, "wo": wo,
        **consts,
    }


_NC_CACHE = {}


def get_nc():
    if "nc" not in _NC_CACHE:
        _NC_CACHE["nc"] = build_nc()
    return _NC_CACHE["nc"]


def kernel(x, Wq, Wk, Wv, Wo):
    x = np.asarray(x, dtype=F32)
    Wq = np.asarray(Wq, dtype=F32)
    Wk = np.asarray(Wk, dtype=F32)
    Wv = np.asarray(Wv, dtype=F32)
    Wo = np.asarray(Wo, dtype=F32)
    nc = get_nc()
    consts = host_constants()
    in_maps = [make_in_map(consts, x, Wq, Wk, Wv, Wo, c // 4, c % 4)
               for c in range(8)]
    res = run_bass_kernel_spmd(nc, in_maps, list(range(8)))
    outs = [r["y"].astype(np.float64) for r in res.results]
    y = np.stack([sum(outs[0:4]), sum(outs[4:8])], axis=0).astype(F32)
    return y
